# revision 21
# baseline (speedup 1.0000x reference)
"""CSWin attention block Trainium2 kernel.

Sharding: one head per NeuronCore (8 heads / 8 cores). Each core computes
both stripe branches (horizontal + vertical) for its 24 channels, the LePE
depthwise conv + GELU residual, and a partial projection over its 24 z
channels. Host sums the 8 partial projections and adds proj_b.

Kernel math notes:
 - relative-position bias folded into the QK contraction exactly via a
   rank-7 SVD of the 7x7 per-head bias table (contraction 24 -> 31); the
   aug rows are produced by the qkv matmul itself from 7 per-branch
   stripe-phase indicator channels appended to x
 - qkv bias handled via a ones-channel appended to x
 - softmax computed without max subtraction (|scores| < 1 for this problem)
 - softmax denominators come from a ones-column appended to V
 - depthwise 3x3 conv on TensorE as 9 diagonal matmuls over shifted views
   of a zero-padded image tile, row-packed 4 taps at a time (K=96)
 - PE clock: HAM only un-throttles on sustained >=96-active-row matmul
   streams but then stays warm (hysteresis), so dense K>=96 warm-up bursts
   are injected at phase boundaries
"""

import sys

for _p in ("/root/.axon_site/_ro/trn_rl_repo", "/opt/trn_rl_repo"):
    if _p not in sys.path:
        sys.path.append(_p)

import numpy as np
import ml_dtypes

import concourse.bass as bass
import concourse.mybir as mybir
import concourse.tile as tile
from concourse.bass_utils import run_bass_kernel_spmd

BF = ml_dtypes.bfloat16
S = 7
NH = 8
C = 192
HD = C // NH            # 24
SCALE = HD ** -0.5
H = W = 112
NWIN = H // S           # 16
L = S * W               # 784 tokens per window
NCORES = 8
AUG = 31                # 24 qk dims + 7 bias dims
CIN = C + 1 + 2 * S     # x + ones + h-indicators + v-indicators = 207
B0, B1 = 104, 103       # contraction split (both >= 96 for PE warmth)
QKVW = 88               # q+aug @ 0, k+aug @ 32, v @ 64

F32 = mybir.dt.float32
F16 = mybir.dt.float16
BF16 = mybir.dt.bfloat16


def _split_waits(nc):
    """walrus in this container accepts at most ONE sync wait per
    instruction; hoist extras onto NoOps ahead of the instruction."""
    maxw = 1
    for f in nc.m.functions:
        for bb in f.blocks:
            newlist, changed = [], False
            for inst in bb.instructions:
                si = inst.sync_info
                waits = list(si.on_wait) if si and si.on_wait else []
                if len(waits) > maxw:
                    keep, extra = waits[-maxw:], waits[:-maxw]
                    k = 0
                    while extra:
                        chunk, extra = extra[:maxw], extra[maxw:]
                        newlist.append(mybir.InstNoOp(
                            name=f"{inst.name}-wsplit{k}", engine=inst.engine,
                            ins=[], outs=[],
                            sync_info=mybir.SyncInfo(on_wait=chunk, on_update=[])))
                        k += 1
                    inst.sync_info = mybir.SyncInfo(
                        on_wait=keep,
                        on_update=list(si.on_update) if si.on_update else [])
                    changed = True
                newlist.append(inst)
            if changed:
                bb.instructions = newlist


def build_program(nwin=NWIN, exp_func=None, gelu_func=None, split=True):
    """Build the single-core Bass program (head-agnostic; weights arrive
    pre-sliced per core)."""
    if exp_func is None:
        exp_func = mybir.ActivationFunctionType.Exp
    if gelu_func is None:
        gelu_func = mybir.ActivationFunctionType.Gelu

    nc = bass.Bass()

    d_x = nc.dram_tensor("x_aug", [CIN, H * W], BF16, kind="ExternalInput")
    d_wqkv = {b: nc.dram_tensor(f"wqkv_{b}", [CIN, 128], BF16, kind="ExternalInput")
              for b in ("h", "v")}
    d_eye = nc.dram_tensor("eye88", [QKVW, HD], BF16, kind="ExternalInput")
    d_dw = {b: nc.dram_tensor(f"dwdiag_{b}", [128, 9 * 128], BF16, kind="ExternalInput")
            for b in ("h", "v")}
    d_lepeb = {b: nc.dram_tensor(f"lepeb_{b}", [128, 1], F32, kind="ExternalInput")
               for b in ("h", "v")}
    d_wproj = nc.dram_tensor("wproj", [120, 8 * 96], BF16, kind="ExternalInput")
    d_out = nc.dram_tensor("out", [C, H * W], F16, kind="ExternalOutput")
    d_recip = {b: nc.dram_tensor(f"recip_scratch_{b}", [nwin, L], F32)
               for b in ("h", "v")}

    with tile.TileContext(nc) as tc:
        import contextlib
        ctx = contextlib.ExitStack()
        with ctx:
            consts = ctx.enter_context(tc.tile_pool(name="consts", bufs=1))
            imgs = ctx.enter_context(tc.tile_pool(name="imgs", bufs=1))

            # ---- persistent constants ----
            # weights FIRST: the DMA queues drain in issue order, so the 300KB
            # of weights must not sit behind the 5.2MB x stream
            wq_sb = {}
            for b in ("h", "v"):
                wq_sb[b] = (consts.tile([B0, 128], BF16, name=f"wq0{b}", tag=f"wq0{b}"),
                            consts.tile([B1, 128], BF16, name=f"wq1{b}", tag=f"wq1{b}"))
                nc.sync.dma_start(out=wq_sb[b][0], in_=d_wqkv[b][0:B0, :])
                nc.sync.dma_start(out=wq_sb[b][1], in_=d_wqkv[b][B0:CIN, :])
            eye_sb = consts.tile([QKVW, HD], BF16, name="eye", tag="eye")
            nc.sync.dma_start(out=eye_sb, in_=d_eye[:, :])
            dw_sb = {b: consts.tile([128, 9 * 128], BF16, name=f"dw{b}", tag=f"dw{b}")
                     for b in ("h", "v")}
            lepeb_sb = {b: consts.tile([128, 1], F32, name=f"lb{b}", tag=f"lb{b}")
                        for b in ("h", "v")}
            for b in ("h", "v"):
                nc.sync.dma_start(out=dw_sb[b], in_=d_dw[b][:, :])
                nc.sync.dma_start(out=lepeb_sb[b], in_=d_lepeb[b][:, :])
            wp_sb = consts.tile([120, 8 * 96], BF16, name="wp", tag="wp")
            nc.sync.dma_start(out=wp_sb, in_=d_wproj[:, :])

            # one tile per 28-row slab so a window's qkv only waits on its own
            # slab's DMA (tile deps are tracked per tile, not per region);
            # each slab split into 2 channel-halves x 2 sub-slices so packets
            # spread across more DMA engines (assignment is per-instruction)
            xv0 = d_x[0:B0].rearrange("c (a b) -> c a b", a=H)
            xv1 = d_x[B0:CIN].rearrange("c (a b) -> c a b", a=H)
            x0s, x1s = [], []
            for sl in range(4):
                rs = slice(28 * sl, 28 * sl + 28)
                t0 = consts.tile([B0, 28, W], BF16, name=f"x0s{sl}", tag=f"x0s{sl}")
                t1 = consts.tile([B1, 28, W], BF16, name=f"x1s{sl}", tag=f"x1s{sl}")
                nc.sync.dma_start(out=t0[0:52], in_=xv0[0:52, rs, :])
                nc.sync.dma_start(out=t0[52:B0], in_=xv0[52:B0, rs, :])
                nc.sync.dma_start(out=t1[0:52], in_=xv1[0:52, rs, :])
                nc.sync.dma_start(out=t1[52:B1], in_=xv1[52:B1, rs, :])
                x0s.append(t0)
                x1s.append(t1)

            # ---- padded branch images (attention outputs) in flat layout,
            # plus their 4-strip views (strip s = padded rows 28s..28s+30 on
            # partitions 32s..32s+24) for the K=128 packed depthwise conv
            pad = {b: imgs.tile([HD, H + 2, W + 2], BF16, name=f"pad{b}", tag=f"pad{b}")
                   for b in ("h", "v")}
            pad2 = {b: imgs.tile([128, 30, W + 2], BF16, name=f"pad2{b}", tag=f"pad2{b}")
                    for b in ("h", "v")}
            for b in ("h", "v"):
                nc.vector.memset(pad[b], 0.0)
                nc.vector.memset(pad2[b], 0.0)

            def warm_burst(name, n=12):
                # dense 104x128-cell matmul burst (81% array occupancy):
                # trips the HAM activity detector into the 2.4 GHz state
                with tc.tile_pool(name=name, bufs=1, space="PSUM") as wpool:
                    t = wpool.tile([128, 448], F32, name=name)
                    xf = x0s[0].rearrange("c a b -> c (a b)")
                    for _ in range(n):
                        nc.tensor.matmul(t, xf[:, 0:128], xf[:, 128:576],
                                         start=True, stop=True)

            # ================= attention (both branches) =================
            for b in ("h", "v"):
                warm_burst(f"warm_{b}")
                with contextlib.ExitStack() as bctx:
                    work = bctx.enter_context(tc.tile_pool(name=f"work_{b}", bufs=1))
                    epool = bctx.enter_context(tc.tile_pool(name=f"e_{b}", bufs=3))
                    attn_pool = bctx.enter_context(tc.tile_pool(name=f"attn_{b}", bufs=nwin))
                    npool = bctx.enter_context(tc.tile_pool(name=f"n_{b}", bufs=1))
                    ps_qkv = bctx.enter_context(
                        tc.tile_pool(name=f"psq_{b}", bufs=1, space="PSUM"))
                    ps_sv = bctx.enter_context(
                        tc.tile_pool(name=f"pssv_{b}", bufs=2, space="PSUM"))
                    ps_av = bctx.enter_context(
                        tc.tile_pool(name=f"psav_{b}", bufs=1, space="PSUM"))

                    sums_sb = npool.tile([nwin, L], BF16, name="sums", tag="sums")
                    attn_tiles = []

                    # persistent double-buffered qkv/ka/vaug tiles, all padded
                    # to KP=124 partitions with zero rows: the PE HAM activity
                    # monitor only counts matmuls with >=96 active rows toward
                    # the 2.4 GHz un-throttle, so the QK contraction (31 live
                    # dims) is zero-padded to 124. Zero lhs rows x finite rhs
                    # rows contribute nothing; the one-time memsets below keep
                    # every padded row finite. The V ones-column is
                    # initialized once.
                    KP = 124
                    qkv_t, ka_t, va_t, eT_t = [], [], [], []
                    for i in range(2):
                        qkv_t.append(work.tile([KP, L], BF16, name=f"qkv{i}", tag=f"qkv{i}"))
                        ka_t.append(work.tile([KP, L], BF16, name=f"ka{i}", tag=f"ka{i}"))
                        va_t.append(work.tile([KP, S, 128], BF16, name=f"va{i}", tag=f"va{i}"))
                        nc.vector.memset(qkv_t[i], 0.0)
                        nc.vector.memset(ka_t[i], 0.0)
                        nc.vector.memset(va_t[i], 0.0)
                        nc.vector.memset(va_t[i][0:W, :, HD:HD + 1], 1.0)
                    for i in range(3):
                        eT_t.append(work.tile([KP, L], BF16, name=f"eT{i}", tag=f"eT{i}"))
                        nc.vector.memset(eT_t[i], 0.0)

                    for w in range(nwin):
                        # -- qkv+aug projection: psum (88, 1024). h windows
                        # read one 7-row strip of slab w//4; v windows read a
                        # 7-col strip of each of the 4 slabs (196 px chunks at
                        # 256-aligned psum cols so matmuls stay in-bank).
                        pq = ps_qkv.tile([128, 1024], F32, name="pqkv", tag="qkv")
                        if b == "h":
                            s, r0 = w // 4, 7 * (w % 4)
                            chunks = [
                                (0, 448, x0s[s][:, r0:r0 + 4, :],
                                 x1s[s][:, r0:r0 + 4, :]),
                                (512, 336, x0s[s][:, r0 + 4:r0 + 7, :],
                                 x1s[s][:, r0 + 4:r0 + 7, :]),
                            ]
                            copies = [(0, 448, 0), (448, 336, 512)]
                        else:
                            cs = slice(7 * w, 7 * w + 7)
                            chunks = [(256 * c, 196, x0s[c][:, :, cs],
                                       x1s[c][:, :, cs]) for c in range(4)]
                            copies = [(196 * c, 196, 256 * c) for c in range(4)]
                        # chunk-outer, K-pass-inner: a start=True matmul resets
                        # has_written bank-wide, so each chunk's accumulation
                        # must complete before the next chunk starts in-bank
                        for col, n, r0c, r1c in chunks:
                            for blk in range(2):
                                nc.tensor.matmul(pq[:, col:col + n], wq_sb[b][blk],
                                                 (r0c, r1c)[blk],
                                                 start=(blk == 0), stop=(blk == 1))

                        # -- single copy to SBUF bf16 (aug rows included);
                        # k half moved to its own base-0 tile by DMA
                        qkv_sb, ka_sb, vaug = qkv_t[w % 2], ka_t[w % 2], va_t[w % 2]
                        for dst, n, src in copies:
                            nc.vector.tensor_copy(out=qkv_sb[0:QKVW, dst:dst + n],
                                                  in_=pq[0:QKVW, src:src + n])
                        nc.sync.dma_start(out=ka_sb[0:AUG, :], in_=qkv_sb[32:63, :])
                        Qa = qkv_sb[0:KP, :]
                        vT = qkv_sb[64:88, :]

                        # -- V transpose into (112, 7, 24) bf16 psum
                        pvt = ps_sv.tile([W, S, HD], BF16, name="pvt", tag="sv")
                        for j in range(S):
                            nc.tensor.transpose(pvt[:, j, :],
                                                vT[:, 112 * j:112 * j + 112],
                                                eye_sb[64:88, :])
                        nc.vector.tensor_copy(out=vaug[0:W, :, 0:HD], in_=pvt)

                        # -- attention over m-blocks (K padded to 124)
                        pav = ps_av.tile([128, L], F32, name="pav", tag="av")
                        for j in range(S):
                            ps = ps_sv.tile([W, L], F32, name="ps_s", tag="sv")
                            nc.tensor.matmul(ps[:, 0:512],
                                             ka_sb[0:KP, 112 * j:112 * j + 112],
                                             Qa[:, 0:512], start=True, stop=True)
                            nc.tensor.matmul(ps[:, 512:L],
                                             ka_sb[0:KP, 112 * j:112 * j + 112],
                                             Qa[:, 512:L], start=True, stop=True)
                            eT = eT_t[j % 3]
                            nc.scalar.activation(out=eT[0:W, :], in_=ps, func=exp_func)
                            nc.tensor.matmul(pav[:, 0:512], vaug[0:KP, j, :],
                                             eT[0:KP, 0:512], start=(j == 0),
                                             stop=(j == S - 1))
                            nc.tensor.matmul(pav[:, 512:L], vaug[0:KP, j, :],
                                             eT[0:KP, 512:L], start=(j == 0),
                                             stop=(j == S - 1))

                        at = attn_pool.tile([HD + 1, L], BF16, name="attn", tag="attn")
                        nc.vector.tensor_copy(out=at, in_=pav[0:HD + 1, :])
                        attn_tiles.append(at)
                        nc.sync.dma_start(out=sums_sb[w:w + 1, :], in_=at[HD:HD + 1, :])

                    # -- normalize: reciprocal once, broadcast via DRAM
                    recip_sb = npool.tile([nwin, L], F32, name="recip", tag="recip")
                    # normalize: 4-deep prefetch of the broadcast DMAs on the
                    # sync queue; multiplies alternate DVE/GpSimd so neither
                    # queue serializes the phase or blocks the next branch
                    nc.vector.reciprocal(out=recip_sb, in_=sums_sb)
                    nc.sync.dma_start(out=d_recip[b][:, :], in_=recip_sb)
                    rrep_t = [npool.tile([HD, L], F32, name=f"rrep{i}",
                                         tag=f"rrep{i}") for i in range(8)]

                    def norm_mult(w, rrep):
                        if b == "h":
                            dst = pad[b][0:HD, 1 + 7 * w:8 + 7 * w, 1:1 + W]
                            a = S
                        else:
                            dst = pad[b][0:HD, 1:1 + H, 1 + 7 * w:8 + 7 * w]
                            a = H
                        i0 = attn_tiles[w][0:HD, :].rearrange(
                            "p (a c) -> p a c", a=a)
                        i1 = rrep.rearrange("p (a c) -> p a c", a=a)
                        eng = nc.vector if w % 2 == 0 else nc.gpsimd
                        eng.tensor_tensor(out=dst, in0=i0, in1=i1,
                                          op=mybir.AluOpType.mult)

                    for w in range(nwin):
                        src = bass.AP(tensor=d_recip[b], offset=w * L,
                                      ap=[[0, HD], [1, L]])
                        nc.sync.dma_start(out=rrep_t[w % 8], in_=src)
                        if w >= 7:
                            norm_mult(w - 7, rrep_t[(w - 7) % 8])
                    for w in range(nwin - 7, nwin):
                        norm_mult(w, rrep_t[w % 8])
                    # 4-strip copies: strip s (partitions 32s..32s+24) holds
                    # padded rows 28s..28s+30 of this branch's image, so one
                    # K=128 block-diag matmul covers 4 row-regions at once
                    # (halo rows come free from the flat pad image)
                    for s4 in range(4):
                        nc.sync.dma_start(
                            out=pad2[b][32 * s4:32 * s4 + HD, :, :],
                            in_=pad[b][0:HD, 28 * s4:28 * s4 + 30, :])

            # ================= LePE (gelu(dw3x3)+residual) ===============
            # strip-packed: each region j covers rows {28s+4j..+4 | s<4} in
            # one (128, 448) psum tile; 9 diagonal taps accumulate with K=128
            # (keeps the HAM un-throttled), gelu lands in strip layout, and
            # the projection contracts each strip block of z with a
            # block-diagonal wp (K=120)
            with contextlib.ExitStack() as lctx:
                gpool = lctx.enter_context(tc.tile_pool(name="g", bufs=1))
                zpool = lctx.enter_context(tc.tile_pool(name="z", bufs=2))
                opool = lctx.enter_context(tc.tile_pool(name="o", bufs=2))

                g2 = {b: gpool.tile([128, S, 448], BF16, name=f"g{b}", tag=f"g{b}")
                      for b in ("h", "v")}
                taps = [(dy, dx) for dy in range(3) for dx in range(3)]
                with tc.tile_pool(name="psdw", bufs=2, space="PSUM") as ps_dw, \
                     tc.tile_pool(name="psp", bufs=2, space="PSUM") as ps_p:
                    # all h regions first: they only need pad2_h, so the PE
                    # stays busy while the v-branch normalize finishes
                    def dw_region(b, j):
                        pdw = ps_dw.tile([128, 448], F32, name="pdw", tag="dw")
                        for t, (dy, dx) in enumerate(taps):
                            rhs = pad2[b][0:128, 4 * j + dy:4 * j + dy + 4,
                                          dx:dx + W]
                            nc.tensor.matmul(
                                pdw, dw_sb[b][:, 128 * t:128 * (t + 1)],
                                rhs, start=(t == 0), stop=(t == 8))
                        nc.scalar.activation(
                            out=g2[b][:, j, :], in_=pdw,
                            func=gelu_func, bias=lepeb_sb[b])

                    for j in range(S):
                        dw_region("h", j)
                    for j in range(S):
                        dw_region("v", j)

                        # combine both branches for region j (all 4 strips)
                        z = zpool.tile([128, 448], BF16, name="z", tag="z")
                        rows = slice(4 * j + 1, 4 * j + 5)
                        nc.vector.tensor_tensor(
                            out=z, in0=pad2["h"][0:128, rows, 1:1 + W],
                            in1=g2["h"][:, j, :], op=mybir.AluOpType.add)
                        nc.vector.tensor_tensor(
                            out=z, in0=z, in1=pad2["v"][0:128, rows, 1:1 + W],
                            op=mybir.AluOpType.add)
                        nc.vector.tensor_tensor(
                            out=z, in0=z, in1=g2["v"][:, j, :],
                            op=mybir.AluOpType.add)

                        # projection: per (strip, out-half) with block-diag wp
                        ot = opool.tile([128, 2, 4, 448], F16, name="ot", tag="ot")
                        for hf in range(2):
                            for s4 in range(4):
                                pp = ps_p.tile([96, 448], F32, name="pp", tag="pp")
                                nc.tensor.matmul(
                                    pp, wp_sb[0:120, (4 * hf + s4) * 96:
                                              (4 * hf + s4 + 1) * 96],
                                    z[0:120, :], start=True, stop=True)
                                nc.vector.tensor_copy(out=ot[0:96, hf, s4, :],
                                                      in_=pp)
                            nc.sync.dma_start(
                                out=bass.AP(tensor=d_out,
                                            offset=96 * hf * (H * W) + 4 * j * W,
                                            ap=[[H * W, 96], [28 * W, 4],
                                                [1, 448]]),
                                in_=ot[0:96, hf, :, :])

    if split:
        _split_waits(nc)
    return nc


def _rel_idx():
    idx = np.arange(S)
    return idx[:, None] - idx[None, :] + S - 1


def prepare_inputs(inputs):
    """Host-side prep: per-core input maps (head h -> core h)."""
    x = np.asarray(inputs["x"], np.float32)[0].reshape(C, H * W)
    px = np.arange(H * W)
    ind_h = (px // W % S == np.arange(S)[:, None]).astype(np.float32)
    ind_v = (px % W % S == np.arange(S)[:, None]).astype(np.float32)
    x_aug = np.concatenate(
        [x, np.ones((1, H * W), np.float32), ind_h, ind_v], 0).astype(BF)

    rel = _rel_idx()
    tbl_h = np.asarray(inputs["bias_table_h"], np.float32)
    tbl_v = np.asarray(inputs["bias_table_v"], np.float32)
    bias_tab = {"h": tbl_h[rel, S - 1, :].transpose(2, 0, 1),
                "v": tbl_v[S - 1, :, :][rel].transpose(2, 0, 1)}
    ind_rows = {"h": C + 1, "v": C + 1 + S}

    qkv_w = {"h": np.asarray(inputs["qkv_h_w"], np.float32),
             "v": np.asarray(inputs["qkv_v_w"], np.float32)}
    qkv_b = {"h": np.asarray(inputs["qkv_h_b"], np.float32),
             "v": np.asarray(inputs["qkv_v_b"], np.float32)}
    lepe_w = {"h": np.asarray(inputs["lepe_h_w"], np.float32),
              "v": np.asarray(inputs["lepe_v_w"], np.float32)}
    lepe_b = {"h": np.asarray(inputs["lepe_h_b"], np.float32),
              "v": np.asarray(inputs["lepe_v_b"], np.float32)}
    proj_w = np.asarray(inputs["proj_w"], np.float32)

    eye88 = np.zeros((QKVW, HD), np.float32)
    eye88[64:88] = np.eye(HD)
    taps = [(dy, dx) for dy in range(3) for dx in range(3)]

    in_maps = []
    for head in range(NCORES):
        m = {"x_aug": x_aug, "eye88": eye88.astype(BF)}
        cs = slice(head * HD, (head + 1) * HD)
        for b in ("h", "v"):
            wa = np.zeros((CIN, 128), np.float32)
            for s3, (c0, scale) in enumerate(
                    [(0, SCALE), (32, 1.0), (64, 1.0)]):
                rows = slice(s3 * C + head * HD, s3 * C + (head + 1) * HD)
                wa[0:C, c0:c0 + HD] = qkv_w[b][rows].T * scale
                wa[C, c0:c0 + HD] = qkv_b[b][rows] * scale
            U, sv, Vt = np.linalg.svd(bias_tab[b][head])
            Aq = (U * np.sqrt(sv)[None, :])            # (S, 7)
            Ak = (Vt * np.sqrt(sv)[:, None])           # (7, S)
            r0 = ind_rows[b]
            wa[r0:r0 + S, 24:31] = Aq                  # aug_q[j,l]=Aq[row(l),j]
            wa[r0:r0 + S, 56:63] = Ak.T                # aug_k[j,m]=Ak[j,row(m)]
            m[f"wqkv_{b}"] = wa.astype(BF)

            # block-diag-of-diagonals: strip i's 24 channels get their own
            # diagonal inside each (128, 128) tap block
            dw = np.zeros((128, 9 * 128), np.float32)
            for t, (dy, dx) in enumerate(taps):
                for i in range(4):
                    r = slice(32 * i, 32 * i + HD)
                    dw[r, 128 * t + 32 * i:128 * t + 32 * i + HD] = np.diag(
                        lepe_w[b][cs, 0, dy, dx])
            m[f"dwdiag_{b}"] = dw.astype(BF)
            lb = np.zeros((128, 1), np.float32)
            for i in range(4):
                lb[32 * i:32 * i + HD, 0] = lepe_b[b][cs]
            m[f"lepeb_{b}"] = lb
        # projection: col block (4*hf + s)*96 holds proj rows 96hf..96hf+96
        # for strip s's 24 channels (rows 32s..32s+24), zeros elsewhere
        wp = np.zeros((120, 8 * 96), np.float32)
        for hf in range(2):
            for s4 in range(4):
                cb = (4 * hf + s4) * 96
                wp[32 * s4:32 * s4 + HD, cb:cb + 96] = \
                    proj_w[96 * hf:96 * hf + 96, cs].T * 0.5
        m["wproj"] = wp.astype(BF)
        in_maps.append(m)
    return in_maps


_NC_CACHE = {}


def get_nc():
    if "nc" not in _NC_CACHE:
        _NC_CACHE["nc"] = build_program()
    return _NC_CACHE["nc"]


def kernel(**inputs):
    nc = get_nc()
    in_maps = prepare_inputs(inputs)
    res = run_bass_kernel_spmd(nc, in_maps, list(range(NCORES)))
    acc = np.zeros((C, H * W), np.float32)
    for r in res.results:
        acc += r["out"].astype(np.float32)
    acc += np.asarray(inputs["proj_b"], np.float32)[:, None]
    return acc.reshape(1, C, H, W).astype(np.float32)



# revision 25
# speedup vs baseline: 1.1285x; 1.1285x over previous
"""CSWin attention block Trainium2 kernel.

Sharding: one head per NeuronCore (8 heads / 8 cores). Each core computes
both stripe branches (horizontal + vertical) for its 24 channels, the LePE
depthwise conv + GELU residual, and a partial projection over its 24 z
channels. Host sums the 8 partial projections and adds proj_b.

Kernel math notes:
 - relative-position bias folded into the QK contraction exactly via a
   rank-7 SVD of the 7x7 per-head bias table (contraction 24 -> 31); the
   aug rows are produced by the qkv matmul itself from 7 per-branch
   stripe-phase indicator channels appended to x
 - qkv bias handled via a ones-channel appended to x
 - softmax computed without max subtraction (|scores| < 1 for this problem)
 - softmax denominators come from a ones-column appended to V
 - depthwise 3x3 conv on TensorE as 9 diagonal matmuls over shifted views
   of a zero-padded image tile, row-packed 4 taps at a time (K=96)
 - PE clock: HAM only un-throttles on sustained >=96-active-row matmul
   streams but then stays warm (hysteresis), so dense K>=96 warm-up bursts
   are injected at phase boundaries
"""

import sys

for _p in ("/root/.axon_site/_ro/trn_rl_repo", "/opt/trn_rl_repo"):
    if _p not in sys.path:
        sys.path.append(_p)

import numpy as np
import ml_dtypes

import concourse.bass as bass
import concourse.mybir as mybir
import concourse.tile as tile
from concourse.bass_utils import run_bass_kernel_spmd

BF = ml_dtypes.bfloat16
S = 7
NH = 8
C = 192
HD = C // NH            # 24
SCALE = HD ** -0.5
H = W = 112
NWIN = H // S           # 16
L = S * W               # 784 tokens per window
NCORES = 8
AUG = 31                # 24 qk dims + 7 bias dims
CIN = C + 1 + 2 * S     # x + ones + h-indicators + v-indicators = 207
B0, B1 = 104, 103       # contraction split (both >= 96 for PE warmth)
QKVW = 88               # q+aug @ 0, k+aug @ 32, v @ 64

F32 = mybir.dt.float32
F16 = mybir.dt.float16
BF16 = mybir.dt.bfloat16


def _split_waits(nc):
    """walrus in this container accepts at most ONE sync wait per
    instruction; hoist extras onto NoOps ahead of the instruction."""
    maxw = 1
    for f in nc.m.functions:
        for bb in f.blocks:
            newlist, changed = [], False
            for inst in bb.instructions:
                si = inst.sync_info
                waits = list(si.on_wait) if si and si.on_wait else []
                if len(waits) > maxw:
                    keep, extra = waits[-maxw:], waits[:-maxw]
                    k = 0
                    while extra:
                        chunk, extra = extra[:maxw], extra[maxw:]
                        newlist.append(mybir.InstNoOp(
                            name=f"{inst.name}-wsplit{k}", engine=inst.engine,
                            ins=[], outs=[],
                            sync_info=mybir.SyncInfo(on_wait=chunk, on_update=[])))
                        k += 1
                    inst.sync_info = mybir.SyncInfo(
                        on_wait=keep,
                        on_update=list(si.on_update) if si.on_update else [])
                    changed = True
                newlist.append(inst)
            if changed:
                bb.instructions = newlist


def build_program(nwin=NWIN, exp_func=None, gelu_func=None, split=True):
    """Build the single-core Bass program (head-agnostic; weights arrive
    pre-sliced per core)."""
    if exp_func is None:
        exp_func = mybir.ActivationFunctionType.Exp
    if gelu_func is None:
        gelu_func = mybir.ActivationFunctionType.Gelu

    nc = bass.Bass()

    d_x = nc.dram_tensor("x_aug", [CIN, H * W], BF16, kind="ExternalInput")
    d_wqkv = {b: nc.dram_tensor(f"wqkv_{b}", [CIN, 128], BF16, kind="ExternalInput")
              for b in ("h", "v")}
    d_eye = nc.dram_tensor("eye88", [QKVW, HD], BF16, kind="ExternalInput")
    d_dw = {b: nc.dram_tensor(f"dwdiag_{b}", [128, 9 * 128], BF16, kind="ExternalInput")
            for b in ("h", "v")}
    d_lepeb = {b: nc.dram_tensor(f"lepeb_{b}", [128, 1], F32, kind="ExternalInput")
               for b in ("h", "v")}
    d_wproj = nc.dram_tensor("wproj", [120, 8 * 96], BF16, kind="ExternalInput")
    d_out = nc.dram_tensor("out", [C, H * W], F16, kind="ExternalOutput")
    d_recip = {b: nc.dram_tensor(f"recip_scratch_{b}", [nwin, L], F32)
               for b in ("h", "v")}

    with tile.TileContext(nc) as tc:
        import contextlib
        ctx = contextlib.ExitStack()
        with ctx:
            consts = ctx.enter_context(tc.tile_pool(name="consts", bufs=1))
            imgs = ctx.enter_context(tc.tile_pool(name="imgs", bufs=1))

            # ---- persistent constants ----
            # weights FIRST: the DMA queues drain in issue order, so the 300KB
            # of weights must not sit behind the 5.2MB x stream
            wq_sb = {}
            for b in ("h", "v"):
                wq_sb[b] = (consts.tile([B0, 128], BF16, name=f"wq0{b}", tag=f"wq0{b}"),
                            consts.tile([B1, 128], BF16, name=f"wq1{b}", tag=f"wq1{b}"))
                nc.sync.dma_start(out=wq_sb[b][0], in_=d_wqkv[b][0:B0, :])
                nc.sync.dma_start(out=wq_sb[b][1], in_=d_wqkv[b][B0:CIN, :])
            eye_sb = consts.tile([QKVW, HD], BF16, name="eye", tag="eye")
            nc.sync.dma_start(out=eye_sb, in_=d_eye[:, :])
            dw_sb = {b: consts.tile([128, 9 * 128], BF16, name=f"dw{b}", tag=f"dw{b}")
                     for b in ("h", "v")}
            lepeb_sb = {b: consts.tile([128, 1], F32, name=f"lb{b}", tag=f"lb{b}")
                        for b in ("h", "v")}
            for b in ("h", "v"):
                nc.sync.dma_start(out=dw_sb[b], in_=d_dw[b][:, :])
                nc.sync.dma_start(out=lepeb_sb[b], in_=d_lepeb[b][:, :])
            wp_sb = consts.tile([120, 8 * 96], BF16, name="wp", tag="wp")
            nc.sync.dma_start(out=wp_sb, in_=d_wproj[:, :])

            # one tile per 28-row slab so a window's qkv only waits on its own
            # slab's DMA (tile deps are tracked per tile, not per region);
            # each slab split into 2 channel-halves x 2 sub-slices so packets
            # spread across more DMA engines (assignment is per-instruction)
            xv0 = d_x[0:B0].rearrange("c (a b) -> c a b", a=H)
            xv1 = d_x[B0:CIN].rearrange("c (a b) -> c a b", a=H)
            x0s, x1s = [], []
            for sl in range(4):
                rs = slice(28 * sl, 28 * sl + 28)
                t0 = consts.tile([B0, 28, W], BF16, name=f"x0s{sl}", tag=f"x0s{sl}")
                t1 = consts.tile([B1, 28, W], BF16, name=f"x1s{sl}", tag=f"x1s{sl}")
                if sl == 0:
                    # 7-row pieces: window w waits only its own rows
                    for q in range(4):
                        qs = slice(7 * q, 7 * q + 7)
                        nc.sync.dma_start(out=t0[:, qs, :], in_=xv0[:, qs, :])
                        nc.sync.dma_start(out=t1[:, qs, :], in_=xv1[:, qs, :])
                else:
                    nc.sync.dma_start(out=t0[0:52], in_=xv0[0:52, rs, :])
                    nc.sync.dma_start(out=t0[52:B0], in_=xv0[52:B0, rs, :])
                    nc.sync.dma_start(out=t1[0:52], in_=xv1[0:52, rs, :])
                    nc.sync.dma_start(out=t1[52:B1], in_=xv1[52:B1, rs, :])
                x0s.append(t0)
                x1s.append(t1)

            # ---- padded branch images (attention outputs) in flat layout,
            # plus their 4-strip views (strip s = padded rows 28s..28s+30 on
            # partitions 32s..32s+24) for the K=128 packed depthwise conv
            pad = {b: imgs.tile([HD, H + 2, W + 2], BF16, name=f"pad{b}", tag=f"pad{b}")
                   for b in ("h", "v")}
            pad2 = {b: imgs.tile([128, 30, W + 2], BF16, name=f"pad2{b}", tag=f"pad2{b}")
                    for b in ("h", "v")}
            for b in ("h", "v"):
                nc.vector.memset(pad[b], 0.0)
                nc.vector.memset(pad2[b], 0.0)

            def warm_burst(name, n=12):
                # dense 104x128-cell matmul burst (81% array occupancy):
                # trips the HAM activity detector into the 2.4 GHz state
                with tc.tile_pool(name=name, bufs=1, space="PSUM") as wpool:
                    t = wpool.tile([128, 448], F32, name=name)
                    xf = x0s[0].rearrange("c a b -> c (a b)")
                    for _ in range(n):
                        nc.tensor.matmul(t, xf[:, 0:128], xf[:, 128:576],
                                         start=True, stop=True)

            # ================= attention (both branches) =================
            for b in ("h", "v"):
                with contextlib.ExitStack() as bctx:
                    work = bctx.enter_context(tc.tile_pool(name=f"work_{b}", bufs=1))
                    epool = bctx.enter_context(tc.tile_pool(name=f"e_{b}", bufs=3))
                    attn_pool = bctx.enter_context(tc.tile_pool(name=f"attn_{b}", bufs=nwin))
                    npool = bctx.enter_context(tc.tile_pool(name=f"n_{b}", bufs=1))
                    ps_qkv = bctx.enter_context(
                        tc.tile_pool(name=f"psq_{b}", bufs=1, space="PSUM"))
                    ps_sv = bctx.enter_context(
                        tc.tile_pool(name=f"pssv_{b}", bufs=2, space="PSUM"))
                    ps_av = bctx.enter_context(
                        tc.tile_pool(name=f"psav_{b}", bufs=1, space="PSUM"))

                    sums_sb = npool.tile([nwin, L], F32, name="sums", tag="sums")
                    attn_tiles = []

                    # persistent double-buffered qkv/ka/vaug tiles, all padded
                    # to KP=124 partitions with zero rows: the PE HAM activity
                    # monitor only counts matmuls with >=96 active rows toward
                    # the 2.4 GHz un-throttle, so the QK contraction (31 live
                    # dims) is zero-padded to 124. Zero lhs rows x finite rhs
                    # rows contribute nothing; the one-time memsets below keep
                    # every padded row finite. The V ones-column is
                    # initialized once.
                    KP = 124
                    qkv_t, ka_t, va_t, eT_t = [], [], [], []
                    for i in range(2):
                        qkv_t.append(work.tile([KP, L], BF16, name=f"qkv{i}", tag=f"qkv{i}"))
                        ka_t.append(work.tile([KP, L], BF16, name=f"ka{i}", tag=f"ka{i}"))
                        va_t.append(work.tile([KP, S, 128], BF16, name=f"va{i}", tag=f"va{i}"))
                        nc.vector.memset(qkv_t[i], 0.0)
                        nc.vector.memset(ka_t[i], 0.0)
                        nc.vector.memset(va_t[i], 0.0)
                        nc.vector.memset(va_t[i][0:W, :, HD:HD + 1], 1.0)
                    for i in range(3):
                        eT_t.append(work.tile([KP, L], BF16, name=f"eT{i}", tag=f"eT{i}"))
                        nc.vector.memset(eT_t[i], 0.0)

                    for w in range(nwin):
                        # -- qkv+aug projection: psum (88, 1024). h windows
                        # read one 7-row strip of slab w//4; v windows read a
                        # 7-col strip of each of the 4 slabs (196 px chunks at
                        # 256-aligned psum cols so matmuls stay in-bank).
                        pq = ps_qkv.tile([128, 1024], F32, name="pqkv", tag="qkv")
                        if b == "h":
                            s, r0 = w // 4, 7 * (w % 4)
                            chunks = [
                                (0, 448, x0s[s][:, r0:r0 + 4, :],
                                 x1s[s][:, r0:r0 + 4, :]),
                                (512, 336, x0s[s][:, r0 + 4:r0 + 7, :],
                                 x1s[s][:, r0 + 4:r0 + 7, :]),
                            ]
                            copies = [(0, 448, 0), (448, 336, 512)]
                        else:
                            cs = slice(7 * w, 7 * w + 7)
                            chunks = [(256 * c, 196, x0s[c][:, :, cs],
                                       x1s[c][:, :, cs]) for c in range(4)]
                            copies = [(196 * c, 196, 256 * c) for c in range(4)]
                        # chunk-outer, K-pass-inner: a start=True matmul resets
                        # has_written bank-wide, so each chunk's accumulation
                        # must complete before the next chunk starts in-bank
                        for col, n, r0c, r1c in chunks:
                            for blk in range(2):
                                nc.tensor.matmul(pq[:, col:col + n], wq_sb[b][blk],
                                                 (r0c, r1c)[blk],
                                                 start=(blk == 0), stop=(blk == 1))

                        # -- single copy to SBUF bf16 (aug rows included);
                        # k half moved to its own base-0 tile by DMA
                        qkv_sb, ka_sb, vaug = qkv_t[w % 2], ka_t[w % 2], va_t[w % 2]
                        for dst, n, src in copies:
                            nc.vector.tensor_copy(out=qkv_sb[0:QKVW, dst:dst + n],
                                                  in_=pq[0:QKVW, src:src + n])
                        nc.sync.dma_start(out=ka_sb[0:AUG, :], in_=qkv_sb[32:63, :])
                        Qa = qkv_sb[0:KP, :]
                        vT = qkv_sb[64:88, :]

                        # -- V transpose into (112, 7, 24) bf16 psum
                        pvt = ps_sv.tile([W, S, HD], BF16, name="pvt", tag="sv")
                        for j in range(S):
                            nc.tensor.transpose(pvt[:, j, :],
                                                vT[:, 112 * j:112 * j + 112],
                                                eye_sb[64:88, :])
                        nc.vector.tensor_copy(out=vaug[0:W, :, 0:HD], in_=pvt)

                        # -- attention over m-blocks (K padded to 124)
                        pav = ps_av.tile([128, L], F32, name="pav", tag="av")
                        for j in range(S):
                            ps = ps_sv.tile([W, L], F32, name="ps_s", tag="sv")
                            nc.tensor.matmul(ps[:, 0:512],
                                             ka_sb[0:KP, 112 * j:112 * j + 112],
                                             Qa[:, 0:512], start=True, stop=True)
                            nc.tensor.matmul(ps[:, 512:L],
                                             ka_sb[0:KP, 112 * j:112 * j + 112],
                                             Qa[:, 512:L], start=True, stop=True)
                            eT = eT_t[j % 3]
                            nc.scalar.activation(out=eT[0:W, :], in_=ps, func=exp_func)
                            nc.tensor.matmul(pav[:, 0:512], vaug[0:KP, j, :],
                                             eT[0:KP, 0:512], start=(j == 0),
                                             stop=(j == S - 1))
                            nc.tensor.matmul(pav[:, 512:L], vaug[0:KP, j, :],
                                             eT[0:KP, 512:L], start=(j == 0),
                                             stop=(j == S - 1))

                        at = attn_pool.tile([HD + 1, L], F32, name="attn", tag="attn")
                        nc.vector.tensor_copy(out=at, in_=pav[0:HD + 1, :])
                        attn_tiles.append(at)
                        nc.sync.dma_start(out=sums_sb[w:w + 1, :], in_=at[HD:HD + 1, :])

                    # -- normalize: reciprocal once, broadcast via DRAM
                    recip_sb = npool.tile([nwin, L], F32, name="recip", tag="recip")
                    # normalize: 4-deep prefetch of the broadcast DMAs on the
                    # sync queue; multiplies alternate DVE/GpSimd so neither
                    # queue serializes the phase or blocks the next branch
                    nc.vector.reciprocal(out=recip_sb, in_=sums_sb)
                    nc.sync.dma_start(out=d_recip[b][:, :], in_=recip_sb)
                    rrep_t = [npool.tile([HD, L], F32, name=f"rrep{i}",
                                         tag=f"rrep{i}") for i in range(4)]

                    def norm_mult(w, rrep):
                        if b == "h":
                            dst = pad[b][0:HD, 1 + 7 * w:8 + 7 * w, 1:1 + W]
                            a = S
                        else:
                            dst = pad[b][0:HD, 1:1 + H, 1 + 7 * w:8 + 7 * w]
                            a = H
                        i0 = attn_tiles[w][0:HD, :].rearrange(
                            "p (a c) -> p a c", a=a)
                        i1 = rrep.rearrange("p (a c) -> p a c", a=a)
                        eng = nc.vector if w % 2 == 0 else nc.gpsimd
                        eng.tensor_tensor(out=dst, in0=i0, in1=i1,
                                          op=mybir.AluOpType.mult)

                    for w in range(nwin):
                        src = bass.AP(tensor=d_recip[b], offset=w * L,
                                      ap=[[0, HD], [1, L]])
                        nc.sync.dma_start(out=rrep_t[w % 4], in_=src)
                        if w >= 3:
                            norm_mult(w - 3, rrep_t[(w - 3) % 4])
                    for w in range(nwin - 3, nwin):
                        norm_mult(w, rrep_t[w % 4])
                    # 4-strip copies: strip s (partitions 32s..32s+24) holds
                    # padded rows 28s..28s+30 of this branch's image, so one
                    # K=128 block-diag matmul covers 4 row-regions at once
                    # (halo rows come free from the flat pad image)
                    for s4 in range(4):
                        nc.sync.dma_start(
                            out=pad2[b][32 * s4:32 * s4 + HD, :, :],
                            in_=pad[b][0:HD, 28 * s4:28 * s4 + 30, :])

            # ================= LePE (gelu(dw3x3)+residual) ===============
            # strip-packed: each region j covers rows {28s+4j..+4 | s<4} in
            # one (128, 448) psum tile; 9 diagonal taps accumulate with K=128
            # (keeps the HAM un-throttled), gelu lands in strip layout, and
            # the projection contracts each strip block of z with a
            # block-diagonal wp (K=120)
            with contextlib.ExitStack() as lctx:
                gpool = lctx.enter_context(tc.tile_pool(name="g", bufs=1))
                zpool = lctx.enter_context(tc.tile_pool(name="z", bufs=2))
                opool = lctx.enter_context(tc.tile_pool(name="o", bufs=2))

                g2 = {b: gpool.tile([128, S, 448], BF16, name=f"g{b}", tag=f"g{b}")
                      for b in ("h", "v")}
                taps = [(dy, dx) for dy in range(3) for dx in range(3)]
                with tc.tile_pool(name="psdw", bufs=2, space="PSUM") as ps_dw, \
                     tc.tile_pool(name="psp", bufs=2, space="PSUM") as ps_p:
                    # all h regions first: they only need pad2_h, so the PE
                    # stays busy while the v-branch normalize finishes
                    def dw_region(b, j):
                        pdw = ps_dw.tile([128, 448], F32, name="pdw", tag="dw")
                        for t, (dy, dx) in enumerate(taps):
                            rhs = pad2[b][0:128, 4 * j + dy:4 * j + dy + 4,
                                          dx:dx + W]
                            nc.tensor.matmul(
                                pdw, dw_sb[b][:, 128 * t:128 * (t + 1)],
                                rhs, start=(t == 0), stop=(t == 8))
                        nc.scalar.activation(
                            out=g2[b][:, j, :], in_=pdw,
                            func=gelu_func, bias=lepeb_sb[b])

                    for j in range(S):
                        dw_region("h", j)
                    for j in range(S):
                        dw_region("v", j)

                        # combine both branches for region j (all 4 strips)
                        z = zpool.tile([128, 448], BF16, name="z", tag="z")
                        rows = slice(4 * j + 1, 4 * j + 5)
                        nc.vector.tensor_tensor(
                            out=z, in0=pad2["h"][0:128, rows, 1:1 + W],
                            in1=g2["h"][:, j, :], op=mybir.AluOpType.add)
                        nc.vector.tensor_tensor(
                            out=z, in0=z, in1=pad2["v"][0:128, rows, 1:1 + W],
                            op=mybir.AluOpType.add)
                        nc.vector.tensor_tensor(
                            out=z, in0=z, in1=g2["v"][:, j, :],
                            op=mybir.AluOpType.add)

                        # projection: per (strip, out-half) with block-diag wp
                        ot = opool.tile([128, 2, 4, 448], F16, name="ot", tag="ot")
                        for hf in range(2):
                            for s4 in range(4):
                                pp = ps_p.tile([96, 448], F32, name="pp", tag="pp")
                                nc.tensor.matmul(
                                    pp, wp_sb[0:120, (4 * hf + s4) * 96:
                                              (4 * hf + s4 + 1) * 96],
                                    z[0:120, :], start=True, stop=True)
                                nc.vector.tensor_copy(out=ot[0:96, hf, s4, :],
                                                      in_=pp)
                            nc.sync.dma_start(
                                out=bass.AP(tensor=d_out,
                                            offset=96 * hf * (H * W) + 4 * j * W,
                                            ap=[[H * W, 96], [28 * W, 4],
                                                [1, 448]]),
                                in_=ot[0:96, hf, :, :])

    if split:
        _split_waits(nc)
    return nc


def _rel_idx():
    idx = np.arange(S)
    return idx[:, None] - idx[None, :] + S - 1


def prepare_inputs(inputs):
    """Host-side prep: per-core input maps (head h -> core h)."""
    x = np.asarray(inputs["x"], np.float32)[0].reshape(C, H * W)
    px = np.arange(H * W)
    ind_h = (px // W % S == np.arange(S)[:, None]).astype(np.float32)
    ind_v = (px % W % S == np.arange(S)[:, None]).astype(np.float32)
    x_aug = np.concatenate(
        [x, np.ones((1, H * W), np.float32), ind_h, ind_v], 0).astype(BF)

    rel = _rel_idx()
    tbl_h = np.asarray(inputs["bias_table_h"], np.float32)
    tbl_v = np.asarray(inputs["bias_table_v"], np.float32)
    bias_tab = {"h": tbl_h[rel, S - 1, :].transpose(2, 0, 1),
                "v": tbl_v[S - 1, :, :][rel].transpose(2, 0, 1)}
    ind_rows = {"h": C + 1, "v": C + 1 + S}

    qkv_w = {"h": np.asarray(inputs["qkv_h_w"], np.float32),
             "v": np.asarray(inputs["qkv_v_w"], np.float32)}
    qkv_b = {"h": np.asarray(inputs["qkv_h_b"], np.float32),
             "v": np.asarray(inputs["qkv_v_b"], np.float32)}
    lepe_w = {"h": np.asarray(inputs["lepe_h_w"], np.float32),
              "v": np.asarray(inputs["lepe_v_w"], np.float32)}
    lepe_b = {"h": np.asarray(inputs["lepe_h_b"], np.float32),
              "v": np.asarray(inputs["lepe_v_b"], np.float32)}
    proj_w = np.asarray(inputs["proj_w"], np.float32)

    eye88 = np.zeros((QKVW, HD), np.float32)
    eye88[64:88] = np.eye(HD)
    taps = [(dy, dx) for dy in range(3) for dx in range(3)]

    in_maps = []
    for head in range(NCORES):
        m = {"x_aug": x_aug, "eye88": eye88.astype(BF)}
        cs = slice(head * HD, (head + 1) * HD)
        for b in ("h", "v"):
            wa = np.zeros((CIN, 128), np.float32)
            for s3, (c0, scale) in enumerate(
                    [(0, SCALE), (32, 1.0), (64, 1.0)]):
                rows = slice(s3 * C + head * HD, s3 * C + (head + 1) * HD)
                wa[0:C, c0:c0 + HD] = qkv_w[b][rows].T * scale
                wa[C, c0:c0 + HD] = qkv_b[b][rows] * scale
            U, sv, Vt = np.linalg.svd(bias_tab[b][head])
            Aq = (U * np.sqrt(sv)[None, :])            # (S, 7)
            Ak = (Vt * np.sqrt(sv)[:, None])           # (7, S)
            r0 = ind_rows[b]
            wa[r0:r0 + S, 24:31] = Aq                  # aug_q[j,l]=Aq[row(l),j]
            wa[r0:r0 + S, 56:63] = Ak.T                # aug_k[j,m]=Ak[j,row(m)]
            m[f"wqkv_{b}"] = wa.astype(BF)

            # block-diag-of-diagonals: strip i's 24 channels get their own
            # diagonal inside each (128, 128) tap block
            dw = np.zeros((128, 9 * 128), np.float32)
            for t, (dy, dx) in enumerate(taps):
                for i in range(4):
                    r = slice(32 * i, 32 * i + HD)
                    dw[r, 128 * t + 32 * i:128 * t + 32 * i + HD] = np.diag(
                        lepe_w[b][cs, 0, dy, dx])
            m[f"dwdiag_{b}"] = dw.astype(BF)
            lb = np.zeros((128, 1), np.float32)
            for i in range(4):
                lb[32 * i:32 * i + HD, 0] = lepe_b[b][cs]
            m[f"lepeb_{b}"] = lb
        # projection: col block (4*hf + s)*96 holds proj rows 96hf..96hf+96
        # for strip s's 24 channels (rows 32s..32s+24), zeros elsewhere
        wp = np.zeros((120, 8 * 96), np.float32)
        for hf in range(2):
            for s4 in range(4):
                cb = (4 * hf + s4) * 96
                wp[32 * s4:32 * s4 + HD, cb:cb + 96] = \
                    proj_w[96 * hf:96 * hf + 96, cs].T * 0.5
        m["wproj"] = wp.astype(BF)
        in_maps.append(m)
    return in_maps


_NC_CACHE = {}


def get_nc():
    if "nc" not in _NC_CACHE:
        _NC_CACHE["nc"] = build_program()
    return _NC_CACHE["nc"]


def kernel(**inputs):
    nc = get_nc()
    in_maps = prepare_inputs(inputs)
    res = run_bass_kernel_spmd(nc, in_maps, list(range(NCORES)))
    acc = np.zeros((C, H * W), np.float32)
    for r in res.results:
        acc += r["out"].astype(np.float32)
    acc += np.asarray(inputs["proj_b"], np.float32)[:, None]
    return acc.reshape(1, C, H, W).astype(np.float32)

